# revision 12
# baseline (speedup 1.0000x reference)
"""DoRA linear layer (nn_DoraLinearLayer) on 8 Trainium2 NeuronCores.

Math: out = (s-1)*(x @ W.T) + 2*s*((x @ A.T) @ B.T),
      s = magnitude / ||W + 2*B@A||_row  (stop-grad norm)

This factors exactly into ONE matmul per token: out = x @ Weff.T with
      cmb.T  = W.T + D0.T,          D0.T = A.T @ (2B).T     (rank-16)
      n2     = colsum(cmb ∘ cmb)                             (row norms²)
      s      = magnitude / sqrt(n2)
      Weff.T = s ∘ cmb.T - W.T      (== (s-1)∘W.T + s∘D0.T)
Device tensors carry a host-side ×16 scale (wt16, b2t16, mag16) so the
squares land in fp8e4m3 range: sq = cmb16² is fp8 and n2 accumulates
via fp8 DoubleRow matmuls (K=256 per instruction); the 1/16 folds into
the PSUM drain (psum = 16·out, drained via tensor_scalar_mul ×1/16).

The PE's HAM clock gate ramps on full-array activity, so every setup
matmul streams full K=128: the rank-16 D0 operands are zero-padded to
128 partitions (cost is column-count, not K) and run back-to-back on a
deep PSUM rotation from the first cycle. The DVE add+square chain
trails the W.T DMA, which is waved across both HWDGE queues.

Sharding: column-parallel over out_features — core i owns rows
[i*512, (i+1)*512) of W/B/magnitude, x and A replicated, output shard
concatenated on the last dim on the host. Host-side work is marshaling
only: casts to fp16, transposes, slicing, static scaling.
"""
import numpy as np

import concourse.bass as bass
import concourse.tile as tile
from concourse import bacc, mybir
from concourse.bass_utils import run_bass_kernel_spmd

N_CORES = 8
TOKENS, D_IN, D_OUT, R = 8192, 4096, 4096, 16
O = D_OUT // N_CORES          # 512 output features per core
P = 128                       # partitions
NCH = D_IN // P               # 32 contraction chunks
SCALING = 2.0                 # lora_alpha / r
SC16 = 16.0                   # static ×16 device scale (fp8 sq range)
N_WARM = 4                    # PE warmup matmuls

# token groups: first is chunk-major so the matmuls trail the weff
# feeder; the rest are tile-major; last is small to shorten the drain
TGROUPS = [(0, 640, True)]
_t = 640
while _t + 512 <= TOKENS - 896:
    TGROUPS.append((_t, 512, False))
    _t += 512
TGROUPS.append((_t, 640, False))
TGROUPS.append((_t + 640, 256, False))

f16 = mybir.dt.float16
f32 = mybir.dt.float32
bf16 = mybir.dt.bfloat16
f8 = mybir.dt.float8e4
Copy = mybir.ActivationFunctionType.Copy

_CACHE: dict = {}


def emit_kernel(nc, tc, xt, wt, a, b2t, mag, out):
    """Emit the per-core program. All DRAM APs are per-core shapes."""
    from contextlib import ExitStack
    from concourse.tile_rust import add_dep_helper

    DoubleRow = mybir.MatmulPerfMode.DoubleRow

    with ExitStack() as ctx:
        singles = ctx.enter_context(tc.tile_pool(name="singles", bufs=1))
        # 8 PSUM banks: gen(7: warm + D0 rotate + s broadcast + main mm)
        # + n2(1, holds the norm accumulator; idle during main)
        ps_gen = ctx.enter_context(tc.tile_pool(name="ps_gen", bufs=7, space="PSUM"))
        ps_n2 = ctx.enter_context(tc.tile_pool(name="ps_n2", bufs=1, space="PSUM"))
        xpool = ctx.enter_context(tc.tile_pool(name="xpool", bufs=2))
        x0pool = ctx.enter_context(tc.tile_pool(name="x0pool", bufs=1))
        opool = ctx.enter_context(tc.tile_pool(name="opool", bufs=4))

        # ---- zero-padded D0 operands: memset full tiles, DMA the real
        # 16 rows on the sync ring first (needed by the first matmuls)
        a_pad = singles.tile([P, D_IN], f16)
        nc.vector.memset(a_pad, 0.0)
        b2t_pad = singles.tile([P, O], f16)
        nc.vector.memset(b2t_pad, 0.0)
        nc.sync.dma_start(out=a_pad[0:R, :], in_=a)
        nc.sync.dma_start(out=b2t_pad[0:R, :], in_=b2t)
        mag_sb = singles.tile([1, O], f32)
        nc.gpsimd.dma_start(out=mag_sb, in_=mag)

        ones8 = singles.tile([P, 2, 16], f8)
        nc.vector.memset(ones8, 1.0)
        ones_row32 = singles.tile([1, P], f32)
        nc.vector.memset(ones_row32, 1.0)

        # pre-warm the ACT Sqrt table so the s-chain doesn't pay the load
        sqrt_warm = singles.tile([1, 1], f32)
        nc.vector.memset(sqrt_warm, 1.0)
        sqrt_warm2 = singles.tile([1, 1], f32)
        nc.scalar.sqrt(sqrt_warm2, sqrt_warm)

        # ---- PE warmup: full-K fp16 matmuls start the HAM clock ramp
        ones128 = singles.tile([P, P], f16)
        nc.vector.memset(ones128, 1.0)
        warm_rhs = singles.tile([P, O], f16)
        nc.vector.memset(warm_rhs, 0.002)
        warm_ps = ps_gen.tile([P, O], f32, name="gen")
        for _ in range(N_WARM):
            nc.tensor.matmul(warm_ps, lhsT=ones128, rhs=warm_rhs,
                             start=True, stop=True)

        # ---- 16·W.T: host-prearranged [p, c, o]; waves alternate
        # between the sync and scalar HWDGE queues for 2x issue rate
        wt_sb = xpool.tile([P, NCH, O], f16, name="xt")
        wt_r = wt.rearrange("p (c o) -> p c o", o=O)
        wt_dmas = []
        wave_edges = [0, 1, 2, 3, 5, 7, 10, 13, 17, 21, 26, NCH]
        for w in range(len(wave_edges) - 1):
            lo, hi = wave_edges[w], wave_edges[w + 1]
            eng = nc.sync if w % 2 == 0 else nc.scalar
            wt_dmas.append(
                eng.dma_start(out=wt_sb[:, lo:hi, :],
                              in_=wt_r[:, lo:hi, :]))
        wt_t = [wt_sb[:, c, :] for c in range(NCH)]

        # ---- setup, trailing the wt DMA:
        #   PE:  D0[c] = A.T_c @ (2B·16).T   (zero-padded to K=128)
        #   DVE: cmb[c] = D0[c] + wt16_c ; sq[c] = cmb[c]²  (fp8)
        #   PE:  n2 += ones @ sq-pair        (fp8 DoubleRow, K=256)
        cmb = singles.tile([P, NCH, O], f16)
        sq_sb = singles.tile([P, NCH, O], f8)
        n2_ps = ps_n2.tile([16, O], f32)
        LAG = 5
        n2_next = 0

        def emit_n2(k):
            nc.tensor.matmul(n2_ps, lhsT=ones8,
                             rhs=sq_sb[:, 2 * k:2 * k + 2, :],
                             perf_mode=DoubleRow,
                             start=(k == 0), stop=(k == NCH // 2 - 1))

        for c in range(NCH):
            d0 = ps_gen.tile([P, O], f32, name="gen")
            nc.tensor.matmul(d0, lhsT=a_pad[:, c * P:(c + 1) * P], rhs=b2t_pad,
                             start=True, stop=True)
            nc.vector.tensor_add(cmb[:, c, :], d0, wt_t[c])
            nc.vector.tensor_mul(sq_sb[:, c, :], cmb[:, c, :], cmb[:, c, :])
            if c >= LAG and (c - LAG) % 2 == 1:
                emit_n2(n2_next)
                n2_next += 1
        for k in range(n2_next, NCH // 2):
            emit_n2(k)

        # ---- prefetch x.T for the first two token groups; chunk DMAs
        # alternate queues, gated until W.T has fully landed on both
        t0, ntok0, _ = TGROUPS[0]
        xt0 = x0pool.tile([P, NCH, ntok0], f16, name="xt0")
        xg0 = xt[:, t0: t0 + ntok0].rearrange("(c p) t -> c p t", p=P)
        gated = {0: False, 1: False}
        for c in range(NCH):
            q = c % 2
            eng = nc.sync if q == 0 else nc.scalar
            dma = eng.dma_start(out=xt0[:, c, :], in_=xg0[c])
            if not gated[q]:
                gated[q] = True
                for wd in wt_dmas:
                    add_dep_helper(dma.ins, wd.ins, True, "x prefetch after wt")
        t1, ntok1, _ = TGROUPS[1]
        xt1 = xpool.tile([P, NCH, ntok1], f16, name="xt")
        xg1 = xt[:, t1: t1 + ntok1].rearrange("(c p) t -> c p t", p=P)
        nc.sync.dma_start(out=xt1, in_=xg1.rearrange("c p t -> p c t"))
        xt_pre = {0: xt0, 1: xt1}

        # ---- s = mag16 / sqrt(n2), broadcast to all partitions ----
        nrm = singles.tile([1, O], f32)
        nc.scalar.sqrt(nrm, n2_ps[0:1, :])
        rn = singles.tile([1, O], f32)
        nc.vector.reciprocal_approx_fast(out=rn, in_=nrm)
        s_row = singles.tile([1, O], f32)
        nc.vector.tensor_mul(s_row, mag_sb, rn)
        s_ps = ps_gen.tile([P, O], f32, name="gen")
        nc.tensor.matmul(s_ps, lhsT=ones_row32, rhs=s_row, start=True, stop=True)

        # ---- Weff16.T = s ∘ cmb16 - wt16, written in place over cmb;
        # trails into the main loop (2 DVE ops/chunk, s read from PSUM)
        weff_t = []
        for c in range(NCH):
            tmp = singles.tile([P, O], f32, name=f"tmp{c % 3}")
            nc.vector.tensor_mul(tmp, cmb[:, c, :], s_ps)
            nc.vector.tensor_sub(cmb[:, c, :], tmp, wt_t[c])
            weff_t.append(cmb[:, c, :])

        # ---- main: psum = x @ Weff16.T, drained ×(1/16) on DVE ----
        for gi, (t0, ntok, chunk_major) in enumerate(TGROUPS):
            nm = ntok // P
            if gi in xt_pre:
                xt_t = xt_pre[gi]
            else:
                xt_t = xpool.tile([P, NCH, ntok], f16, name="xt")
                xg = xt[:, t0: t0 + ntok].rearrange("(c p) t -> c p t", p=P)
                nc.sync.dma_start(out=xt_t, in_=xg.rearrange("c p t -> p c t"))
            if chunk_major:
                # consume each weff chunk nm x as soon as it lands
                pss = [ps_gen.tile([P, O], f32, name="gen") for _ in range(nm)]
                for c in range(NCH):
                    for m in range(nm):
                        nc.tensor.matmul(
                            pss[m],
                            lhsT=xt_t[:, c, m * P: (m + 1) * P],
                            rhs=weff_t[c],
                            start=(c == 0), stop=(c == NCH - 1),
                        )
                for m in range(nm):
                    ot = opool.tile([P, O], f32, name="ot")
                    nc.vector.tensor_scalar_mul(ot, pss[m], 1.0 / SC16)
                    nc.scalar.dma_start(
                        out=out[t0 + m * P: t0 + (m + 1) * P, :], in_=ot)
            else:
                for m in range(nm):
                    ps = ps_gen.tile([P, O], f32, name="gen")
                    for c in range(NCH):
                        nc.tensor.matmul(
                            ps,
                            lhsT=xt_t[:, c, m * P: (m + 1) * P],
                            rhs=weff_t[c],
                            start=(c == 0), stop=(c == NCH - 1),
                        )
                    ot = opool.tile([P, O], f32, name="ot")
                    nc.vector.tensor_scalar_mul(ot, ps, 1.0 / SC16)
                    nc.scalar.dma_start(
                        out=out[t0 + m * P: t0 + (m + 1) * P, :], in_=ot)


def build_nc():
    if "nc" in _CACHE:
        return _CACHE["nc"]
    nc = bacc.Bacc("TRN2", target_bir_lowering=False, debug=False,
                   num_devices=N_CORES)
    xt = nc.dram_tensor("xt", [D_IN, TOKENS], f16, kind="ExternalInput").ap()
    wt = nc.dram_tensor("wt", [P, NCH * O], f16, kind="ExternalInput").ap()
    a = nc.dram_tensor("a", [R, D_IN], f16, kind="ExternalInput").ap()
    b2t = nc.dram_tensor("b2t", [R, O], f16, kind="ExternalInput").ap()
    mag = nc.dram_tensor("mag", [1, O], f32, kind="ExternalInput").ap()
    out = nc.dram_tensor("out", [TOKENS, O], f32, kind="ExternalOutput").ap()
    with tile.TileContext(nc) as tc:
        emit_kernel(nc, tc, xt, wt, a, b2t, mag, out)
    nc.compile()
    _CACHE["nc"] = nc
    return nc


def prep_in_maps(x, lora_A_w, lora_B_w, base_w, magnitude):
    xt_np = np.ascontiguousarray(x.astype(np.float16).T)
    a_np = np.ascontiguousarray(lora_A_w.astype(np.float16))
    in_maps = []
    for c in range(N_CORES):
        sl = slice(c * O, (c + 1) * O)
        # 16·W.T partition-major: wt_dev[p, c*O + o] = 16·W.T[c*128 + p, o]
        wt_sh = np.ascontiguousarray(
            (SC16 * base_w[sl].astype(np.float32)).astype(np.float16).T)
        wt_dev = np.ascontiguousarray(
            wt_sh.reshape(NCH, P, O).transpose(1, 0, 2).reshape(P, NCH * O))
        in_maps.append({
            "xt": xt_np,
            "wt": wt_dev,
            "a": a_np,
            "b2t": np.ascontiguousarray(
                (SC16 * SCALING * lora_B_w[sl].astype(np.float32))
                .astype(np.float16).T),
            "mag": np.ascontiguousarray(
                (SC16 * magnitude[sl]).reshape(1, O).astype(np.float32)),
        })
    return in_maps


def kernel(x, lora_A_w, lora_B_w, base_w, magnitude):
    nc = build_nc()
    in_maps = prep_in_maps(x, lora_A_w, lora_B_w, base_w, magnitude)
    res = run_bass_kernel_spmd(nc, in_maps, list(range(N_CORES)))
    return np.concatenate(
        [res.results[c]["out"] for c in range(N_CORES)], axis=1)
